# revision 21
# baseline (speedup 1.0000x reference)
"""Trainium2 Bass kernel for nn_Attention_81776177315877.

Separable-conv attention block (CMT/PVT style):
  x (B=8, 3136, 256) -> q/k/v = sepconv(dw3x3+BN+pw1x1, k/v stride 2)
  -> 8-head attention (d=32) -> proj.

Sharding: data-parallel over batch, core b <- batch b. No collectives.

v2 design (per core, channel-major layouts):
  - host: x transposed/padded/cast to bf16; BN+dw taps folded into the
    pointwise weights (9 taps x 256 x 256) -> sepconv = 18 PSUM-accumulated
    K=128 matmuls per output block, bias-add on DVE -> bf16 channel-major.
  - attention per (ic, hg, jt): S-wave (4 heads, tile_position row-packed),
    exp split between ACT (native, bf16 out) and DVE (Schraudolph-style
    bitcast fast-exp: i16 = a*S + b reinterpreted as bf16), O/D waves
    col-packed with PSUM accumulation over jt.
  - proj: token-major PE matmuls (lhsT = o_cm data), bias via K=1 matmul,
    PSUM -> DVE copy -> f32 SBUF -> DMA out. No transposes, no scratch.
  - PSUM: 6 banks S-pipeline (3 x [112,2,448]), 1 bank O-accum ring,
    1 bank shared d/conv/proj ring.
"""

import sys

sys.path.insert(0, "/opt/trn_rl_repo")

import numpy as np
import ml_dtypes

import concourse.bass as bass
import concourse.bacc as bacc
import concourse.mybir as mybir
import concourse.tile as tile
from concourse.bass_utils import run_bass_kernel_spmd
from concourse.masks import make_identity

FP = mybir.dt.float32
BF = mybir.dt.bfloat16
F8 = mybir.dt.float8e4
I16 = mybir.dt.int16
AF = mybir.ActivationFunctionType
ALU = mybir.AluOpType
DR = mybir.MatmulPerfMode.DoubleRow

C = 256
HEADS = 8
D = 32
HH = 56
N = HH * HH          # 3136 query tokens
HK = 28
NK = HK * HK         # 784 key tokens
PADW = HH + 2        # 58
EPS = 1e-5
SCALE = D ** -0.5

IC_CH = 8            # query rows per chunk -> 448 free
IC_F = IC_CH * HH    # 448
N_IC = HH // IC_CH   # 7
KC_CH = 14           # k/v output rows per chunk -> 392 free
KC_F = KC_CH * HK    # 392
N_KC = HK // KC_CH   # 2
JT = 112             # key tile (partitions) for attention
N_JT = NK // JT      # 7

WSCALE = 2.0 ** 7    # fp8 conv-weight prescale (descaled in bias add)
# Schraudolph fast exp in bf16 bits: i16 = A_EXP * S + B_EXP, bits -> bf16
A_EXP = 184.6649652 * SCALE
B_EXP = 16250.5

_CACHED = {}


def _build_nc():
    nc = bacc.Bacc("TRN2", target_bir_lowering=False, debug=False, num_devices=8)

    xpad_d = nc.dram_tensor("x_pad", [2, 128, PADW, PADW], BF, kind="ExternalInput")
    w9_d = {}
    const_d = {}
    for p in ("q", "k", "v"):
        # (tap, cin128, cbi, cout)
        w9_d[p] = nc.dram_tensor(f"{p}_w9", [9, 128, 2, C], BF, kind="ExternalInput")
        const_d[p] = nc.dram_tensor(f"{p}_const", [C, 1], FP, kind="ExternalInput")
    pwT_d = nc.dram_tensor("proj_wT", [2, 128, C], BF, kind="ExternalInput")
    pb_d = nc.dram_tensor("proj_brep", [128, C], FP, kind="ExternalInput")
    out_d = nc.dram_tensor("out", [N, C], FP, kind="ExternalOutput")

    with tile.TileContext(nc) as tc:
        with (
            tc.tile_pool(name="persist", bufs=1) as pp,
            tc.tile_pool(name="ep", bufs=6) as ep,
            tc.tile_pool(name="rp", bufs=2) as rp,
            tc.tile_pool(name="op", bufs=3) as otp,
            tc.tile_pool(name="psS", bufs=2, space="PSUM") as psS,
            tc.tile_pool(name="psA", bufs=1, space="PSUM") as psA,
            tc.tile_pool(name="psB", bufs=1, space="PSUM") as psB,
            tc.tile_pool(name="psC", bufs=2, space="PSUM") as psC,
        ):
            ident = pp.tile([128, 128], FP, tag="ident", name="ident")
            make_identity(nc, ident[:])
            ones32 = pp.tile([128, 32], BF, tag="ones32", name="ones32")
            nc.gpsimd.memset(ones32[:], 1.0)


            # ---- load inputs / weights (ordered so k conv can start asap) ----
            x_pad = pp.tile([128, 2, PADW, PADW], BF, tag="xpad", name="xpad")
            nc.sync.dma_start(x_pad[:], xpad_d[:, :, :, :].rearrange("b c h w -> c b h w"))
            w9 = {}
            consts = {}
            for p in ("k", "v", "q"):
                w9[p] = pp.tile([128, 9, 2, C], BF, tag=f"w9{p}", name=f"w9{p}")
                consts[p] = [
                    pp.tile([128, 1], FP, tag=f"const_{p}{cb}", name=f"const_{p}{cb}")
                    for cb in range(2)
                ]

            def load_w(p):
                nc.sync.dma_start(
                    w9[p][:], w9_d[p][:, :, :, :].rearrange("t c b o -> c t b o")
                )
                for cb in range(2):
                    nc.sync.dma_start(consts[p][cb][:], const_d[p][cb * 128:(cb + 1) * 128, :])

            load_w("k")
            load_w("q")
            load_w("v")
            pwT = [pp.tile([128, C], BF, tag=f"pwT{cb}", name=f"pwT{cb}") for cb in range(2)]
            for cb in range(2):
                nc.sync.dma_start(pwT[cb][:], pwT_d[cb, :, :])
            pb_rep = pp.tile([128, C], FP, tag="pbrep", name="pbrep")
            nc.sync.dma_start(pb_rep[:], pb_d[:, :])

            xp2 = x_pad[:].rearrange("p b (ho a) (wv c) -> p b ho a wv c", a=2, c=2)

            # ---- conv helper: 9-tap folded sepconv, K=128 matmuls ----
            # Emits as closures (3 taps each) so chunks can interleave with
            # the attention loop's PE waves; psC ring holds the accumulators.
            def conv_chunk_ops(p, dst_tiles, stride, ch_rows, wo, ch_idx):
                fsz = ch_rows * wo
                ops = []
                for half in range(2):  # couts [0:128), [128:256)
                    cell = {}

                    def grp(t0, t1, half=half, cell=cell, p=p, ch_idx=ch_idx,
                            fsz=fsz, stride=stride, ch_rows=ch_rows, wo=wo):
                        if t0 == 0:
                            cell["ps"] = psC.tile([128, 448], FP, tag="c", name="cps")
                        cps = cell["ps"]
                        for tap in range(t0, t1):
                            dh, dw = tap // 3 - 1, tap % 3 - 1
                            r0 = 1 + stride * ch_idx * ch_rows + dh
                            c0 = 1 + dw
                            for cbi in range(2):
                                if stride == 1:
                                    rhs = x_pad[:, cbi, r0:r0 + ch_rows, c0:c0 + wo]
                                else:
                                    rhs = xp2[
                                        :, cbi,
                                        r0 // 2: r0 // 2 + ch_rows, r0 % 2,
                                        c0 // 2: c0 // 2 + wo, c0 % 2,
                                    ]
                                nc.tensor.matmul(
                                    cps[:, :fsz],
                                    lhsT=(w9[p][:, tap, cbi, half * 128:(half + 1) * 128]),
                                    rhs=(rhs),
                                    start=(tap == 0 and cbi == 0),
                                    stop=(tap == 8 and cbi == 1),
                                )
                        if t1 == 9:
                            nc.vector.tensor_scalar_add(
                                dst_tiles[half][:, ch_idx * fsz:(ch_idx + 1) * fsz],
                                cps[:, :fsz],
                                consts[p][half],
                            )

                    for t0 in range(0, 9, 3):
                        ops.append(lambda t0=t0, g=grp: g(t0, t0 + 3))
                return ops

            def emit_all(ops):
                for op in ops:
                    op()

            # ---- k conv first, then q0, then v (v needed only at first O) ----
            k_cm = [pp.tile([128, NK], BF, tag=f"kcm{cb}", name=f"kcm{cb}") for cb in range(2)]
            v_cm = [pp.tile([128, NK], FP, tag=f"vcm{cb}", name=f"vcm{cb}") for cb in range(2)]
            q_cm = [pp.tile([128, N], BF, tag=f"qcm{cb}", name=f"qcm{cb}") for cb in range(2)]
            o_cm = [pp.tile([128, N], BF, tag=f"ocm{cb}", name=f"ocm{cb}") for cb in range(2)]

            for ch in range(N_KC):
                emit_all(conv_chunk_ops("k", k_cm, 2, KC_CH, HK, ch))
            emit_all(conv_chunk_ops("q", q_cm, 1, IC_CH, HH, 0))
            for ch in range(N_KC):
                emit_all(conv_chunk_ops("v", v_cm, 2, KC_CH, HK, ch))
            v_tm = pp.tile([128, N_JT, 2, 128], BF, tag="vtm", name="vtm")
            for jt in range(N_JT):
                for cb in range(2):
                    tp = (psA if cb == 0 else psB).tile(
                        [128, 448], FP, tag=f"ps{'AB'[cb]}", name="tp"
                    )
                    nc.tensor.transpose(
                        tp[:JT, :128],
                        v_cm[cb][:, jt * JT:(jt + 1) * JT],
                        ident[:],
                    )
                    nc.vector.tensor_copy(v_tm[:JT, jt, cb, :], tp[:JT, :128])

            # ---- proj block emitter (token-major, PSUM -> DVE -> DMA) ----
            def proj_block(st):
                def op():
                    pps = psC.tile([128, 448], FP, tag="c", name="pj")
                    for cb in range(2):
                        nc.tensor.matmul(
                            pps[:, :C],
                            lhsT=(o_cm[cb][:, st:st + 128]),
                            rhs=(pwT[cb][:, :]),
                            start=(cb == 0),
                            stop=(cb == 1),
                        )
                    ot = otp.tile([128, C], FP, tag="ot", name="ot")
                    nc.vector.scalar_tensor_tensor(
                        ot[:], pps[:, :C], 1.0, pb_rep[:], ALU.mult, ALU.add
                    )
                    nc.sync.dma_start(out_d[st:st + 128, :], ot[:])
                return op

            # ---- main loop: attention with conv/proj interleaved as bg work ----
            from collections import deque
            bg = deque()
            n_tt = (N + 127) // 128  # 25 output token blocks
            ti_done = 0
            ti_ready = 0

            for ic in range(N_IC):
                # enqueue proj blocks covered by last chunk, then next q conv
                while ti_ready < n_tt and min(ti_ready * 128, N - 128) + 128 <= ic * IC_F:
                    bg.append(proj_block(min(ti_ready * 128, N - 128)))
                    ti_ready += 1
                if ic + 1 < N_IC:
                    bg.extend(conv_chunk_ops("q", q_cm, 1, IC_CH, HH, ic + 1))

                for hg in range(2):
                    o_ps = psA.tile([128, 448], FP, tag="psA", name="o")
                    d_ps = psB.tile([128, 448], FP, tag="psB", name="d")

                    def s_pair(jt, p2):
                        s4p = psS.tile([112, 2, 512], FP, tag="s4", name="s4")
                        for hh in (2 * p2, 2 * p2 + 1):
                            nc.tensor.matmul(
                                s4p[:JT, hh % 2, :IC_F],
                                lhsT=(k_cm[hg][hh * 32:(hh + 1) * 32, jt * JT:(jt + 1) * JT]),
                                rhs=(q_cm[hg][hh * 32:(hh + 1) * 32, ic * IC_F:(ic + 1) * IC_F]),
                                start=True,
                                stop=True,
                                tile_position=(32 * hh, 0),
                                skip_group_check=True,
                            )
                        return s4p

                    s4s = [s_pair(0, 0), s_pair(0, 1)]
                    for jt in range(N_JT):
                        e4p = [None, None]
                        for p2 in range(2):
                            e4 = ep.tile([112, 2, 448], BF, tag="e", name="e")
                            # p2=0 -> ACT; p2=1 -> DVE, except every 4th jt
                            # both go to ACT (keeps DVE free for its other work)
                            if p2 == 0 or (jt + ic) % 4 == 3:
                                nc.scalar.activation(
                                    e4[:JT, :, :], s4s[p2][:JT, :, :IC_F], AF.Exp, scale=SCALE
                                )
                            else:
                                nc.vector.tensor_scalar(
                                    e4[:JT, :, :].bitcast(I16),
                                    s4s[p2][:JT, :, :IC_F],
                                    A_EXP,
                                    B_EXP,
                                    ALU.mult,
                                    ALU.add,
                                )
                            e4p[p2] = e4
                        if jt + 1 < N_JT:
                            s4s = [s_pair(jt + 1, 0), s_pair(jt + 1, 1)]
                        for p2 in range(2):
                            for hh in (2 * p2, 2 * p2 + 1):
                                nc.tensor.matmul(
                                    o_ps[hh * 32:(hh + 1) * 32, :],
                                    lhsT=(v_tm[:JT, jt, hg, hh * 32:(hh + 1) * 32]),
                                    rhs=(e4p[p2][:JT, hh % 2, :]),
                                    start=(jt == 0),
                                    stop=(jt == N_JT - 1),
                                    tile_position=(0, 32 * hh),
                                    skip_group_check=True,
                                )
                        for p2 in range(2):
                            for hh in (2 * p2, 2 * p2 + 1):
                                nc.tensor.matmul(
                                    d_ps[hh * 32:(hh + 1) * 32, :],
                                    lhsT=(ones32[:JT, :]),
                                    rhs=(e4p[p2][:JT, hh % 2, :]),
                                    start=(jt == 0),
                                    stop=(jt == N_JT - 1),
                                    tile_position=(0, 32 * hh),
                                    skip_group_check=True,
                                )
                        if bg:
                            bg.popleft()()

                    r_t = rp.tile([128, IC_F], FP, tag="r", name="r")
                    nc.vector.reciprocal_approx_fast(r_t[:], d_ps[:])
                    nc.vector.tensor_mul(
                        o_cm[hg][:, ic * IC_F:(ic + 1) * IC_F], o_ps[:], r_t[:]
                    )

            while bg:
                bg.popleft()()
            for ti in range(ti_ready, n_tt):
                proj_block(min(ti * 128, N - 128))()

    nc.compile()
    return nc


def _fold_weights(inp, p):
    scale = inp[f"{p}_bn_g"] / np.sqrt(inp[f"{p}_bn_v"] + EPS)
    shift = inp[f"{p}_bn_b"] - inp[f"{p}_bn_m"] * scale
    w2 = inp[f"{p}_pw_w"] * scale[None, :]          # (cout, cin)
    w9 = inp[f"{p}_dw_w"].reshape(C, 9)             # (cin, tap)
    w9t = w2.T[None, :, :] * w9.T[:, :, None]       # (tap, cin, cout)
    # (tap, cin128, cbi, cout)
    w9b = np.ascontiguousarray(
        w9t.reshape(9, 2, 128, C).transpose(0, 2, 1, 3)
    ).astype(ml_dtypes.bfloat16)
    const = (
        inp[f"{p}_pw_w"] @ (scale * inp[f"{p}_dw_b"] + shift) + inp[f"{p}_pw_b"]
    ).astype(np.float32)
    return w9b, const.reshape(C, 1)


def prepare_common(inp):
    common = {}
    for p in ("q", "k", "v"):
        w9b, const = _fold_weights(inp, p)
        common[f"{p}_w9"] = w9b
        common[f"{p}_const"] = const
    common["proj_wT"] = np.ascontiguousarray(
        inp["proj_w"].T.reshape(2, 128, C)
    ).astype(ml_dtypes.bfloat16)
    common["proj_brep"] = np.ascontiguousarray(
        np.broadcast_to(inp["proj_b"].reshape(1, C), (128, C))
    ).astype(np.float32)
    return common


def prepare_x(xb):
    # xb: (3136, 256) f32 -> padded channel-major bf16 (2, 128, 58, 58)
    xt = xb.T.reshape(C, HH, HH)
    xp = np.zeros((C, PADW, PADW), np.float32)
    xp[:, 1:57, 1:57] = xt
    return np.ascontiguousarray(xp.reshape(2, 128, PADW, PADW)).astype(
        ml_dtypes.bfloat16
    )


def prepare_in_maps(inp):
    common = prepare_common(inp)
    x = inp["x"].astype(np.float32)
    return [dict(common, x_pad=prepare_x(x[b])) for b in range(x.shape[0])]


def kernel(**inputs):
    inp = {k: np.asarray(v) for k, v in inputs.items()}

    if "nc" not in _CACHED:
        _CACHED["nc"] = _build_nc()
    nc = _CACHED["nc"]

    in_maps = prepare_in_maps(inp)
    res = run_bass_kernel_spmd(nc, in_maps, list(range(len(in_maps))))
    out = np.stack([res.results[b]["out"] for b in range(len(in_maps))], axis=0)
    return out.astype(np.float32)
